# revision 1
# baseline (speedup 1.0000x reference)
"""Multi-head self-attention (B=2, N=2048, D=1024, H=16) on 8 Trainium2 cores.

Sharding: core c -> batch b = c // 4, head group g = c % 4 (heads 4g..4g+3).
Each core computes q/k/v for its 4 heads, attention in transposed layout
(scores^T = [j, i] so no big transposes are needed), and a partial output
projection restricted to its heads' rows of w_proj.  The host transposes x
once per batch on the way in, and sums the 4 per-core partial projections
(+ residual x) per batch on the way out.

Per-core kernel layout (matmul operands float32r, fp32 elsewhere):
  P1  qkv:   qT/kT per head-pair [128=2*64 dims, 2048 tok], v [tok, 256]
  P2  attn:  per pair, per i-tile(512), per j-chunk(128):
               scores^T row-packed pair (K=64 x2) -> psum [128, 1024]
               exp via one ACT activate (scale=1/8), no max subtraction
               PV col-packed (M=32 x4) + denominator rows in spare col slots
             then reciprocal + PE-broadcast + DVE normalize -> outT [dd, tok]
  P3  proj:  partial^T [1024, 2048] = wp^T-chunks @ outT, DVE evac, DMA out
"""

import numpy as np

import concourse.bass as bass
import concourse.bacc as bacc
import concourse.mybir as mybir
import concourse.tile as tile
from concourse.bass_utils import run_bass_kernel_spmd

B = 2
N = 2048
D = 1024
NH = 16
DH = 64
N_CORES = 8
TP = 4                # head-parallel ways per batch
HPC = NH // TP        # heads per core
HDIM = HPC * DH       # 256 head dims per core
PAIRS = HPC // 2
SCALE = 1.0 / 8.0     # 1/sqrt(DH)

IT = N // 512         # 4 i-tiles
JT = N // 128         # 16 j-chunks
KC = D // 128         # 8 feature chunks

F32 = mybir.dt.float32
F32R = mybir.dt.float32r
AF = mybir.ActivationFunctionType


def build_bass():
    nc = bacc.Bacc("TRN2", target_bir_lowering=False, debug=False)
    xT = nc.declare_dram_parameter("xT", [D, N], F32R, isOutput=False)
    wq = nc.declare_dram_parameter("wq", [D, HDIM], F32R, isOutput=False)
    wk = nc.declare_dram_parameter("wk", [D, HDIM], F32R, isOutput=False)
    wv = nc.declare_dram_parameter("wv", [D, HDIM], F32R, isOutput=False)
    wp = nc.declare_dram_parameter("wp", [HDIM, D], F32R, isOutput=False)
    ones4_d = nc.declare_dram_parameter("ones4_c", [128, 4], F32R, isOutput=False)
    selAB_d = nc.declare_dram_parameter("selAB_c", [128, 128], F32R, isOutput=False)
    zeros_d = nc.declare_dram_parameter("zeros_c", [128, 512], F32R, isOutput=False)
    pT0 = nc.declare_dram_parameter("pT0", [D, N], F32, isOutput=True)
    pT1 = nc.declare_dram_parameter("pT1", [D, N], F32, isOutput=True)

    with tile.TileContext(nc) as tc:
        with (
            tc.tile_pool(name="big", bufs=1) as big,
            tc.tile_pool(name="exps", bufs=3) as exps,
            tc.tile_pool(name="evac", bufs=4) as evac,
            tc.tile_pool(name="psum", bufs=1, space="PSUM") as psum,
        ):
            # ---- constants (DMA'd: memset can't produce f32r) ----
            # selAB[k, m] = 1 iff (k==0, m<64) or (k==64, m>=64): broadcast matmul
            selAB = big.tile([128, 128], F32R, tag="selAB")
            nc.sync.dma_start(out=selAB, in_=selAB_d[:, :])
            # recip_pad rows 1..63 and 65..127 must stay zero; rows 0/64 are
            # rewritten with the softmax reciprocals each i-tile.
            recip_pad = big.tile([128, 512], F32R, tag="recip_pad")
            nc.sync.dma_start(out=recip_pad, in_=zeros_d[:, :])

            # ---- P1: load inputs (per-chunk tiles keep sync-wait fan-in low) ----
            xts, wqs, wks, wvs = [], [], [], []
            for k in range(KC):
                for lst, nm, src_t, w in (
                    (xts, "xt", xT, N), (wqs, "wq", wq, HDIM),
                    (wks, "wk", wk, HDIM), (wvs, "wv", wv, HDIM),
                ):
                    t = big.tile([128, w], F32R, tag=f"{nm}{k}")
                    eng = nc.sync if (k % 2 == 0) else nc.scalar
                    eng.dma_start(out=t, in_=src_t[k * 128:(k + 1) * 128, :])
                    lst.append(t)
            wps = []
            for p in range(PAIRS):
                t = big.tile([128, D], F32R, tag=f"wp{p}")
                nc.sync.dma_start(out=t, in_=wp[p * 128:(p + 1) * 128, :])
                wps.append(t)

            # ---- P1: qT/kT pair 0 now; pair 1 + v interleaved into P2 ----
            qT = big.tile([128, PAIRS, N], F32R, tag="qT")
            kT = big.tile([128, PAIRS, N], F32R, tag="kT")

            def emit_qk_tile(p, w_s, dst, nt):
                ps = psum.tile([128, 512], F32, tag="mm", bufs=2)
                for k in range(KC):
                    nc.tensor.matmul(
                        ps,
                        lhsT=w_s[k][:, p * 128:(p + 1) * 128],
                        rhs=xts[k][:, nt * 512:(nt + 1) * 512],
                        start=(k == 0),
                        stop=(k == KC - 1),
                    )
                nc.vector.tensor_copy(dst[:, p, nt * 512:(nt + 1) * 512], ps)

            for w_s, dst in ((wqs, qT), (wks, kT)):
                for nt in range(IT):
                    emit_qk_tile(0, w_s, dst, nt)

            # v_aug: per head [v_h | ones] = 65 cols; pair p head h at
            # offset (2p+h)*65.  PV matmul lhsT [128, 65] then yields the
            # softmax denominator as output row 64 for free.
            v_s = big.tile([128, JT, 4 * 65], F32R, tag="v")

            def emit_v_chunk(t):
                nc.sync.dma_start(
                    out=v_s[:, t, :].rearrange("p (h c) -> p h c", c=65)[:, :, 64:65],
                    in_=ones4_d[:, :].unsqueeze(2),
                )
                ps = psum.tile([128, HDIM], F32, tag="mm", bufs=2)
                for k in range(KC):
                    nc.tensor.matmul(
                        ps,
                        lhsT=xts[k][:, t * 128:(t + 1) * 128],
                        rhs=wvs[k],
                        start=(k == 0),
                        stop=(k == KC - 1),
                    )
                nc.vector.tensor_copy(
                    v_s[:, t, :].rearrange("p (h c) -> p h c", c=65)[:, :, 0:64],
                    ps.rearrange("p (h c) -> p h c", c=64),
                )

            # ---- P2: attention (transposed flow) ----
            warm = evac.tile([1, 1], F32, tag="warm")
            nc.scalar.activation(warm, selAB[0:1, 0:1], AF.Exp)
            outTn = big.tile([128, PAIRS, N], F32R, tag="outTn")

            def emit_scores(p, it, jt):
                sc = psum.tile([128, 1024], F32, tag="sc", bufs=2)
                for h in range(2):
                    nc.tensor.matmul(
                        sc[:, h * 512:(h + 1) * 512],
                        lhsT=kT[h * 64:(h + 1) * 64, p, jt * 128:(jt + 1) * 128],
                        rhs=qT[h * 64:(h + 1) * 64, p, it * 512:(it + 1) * 512],
                        start=True,
                        stop=True,
                    )
                return sc

            def emit_proj_tile(p, dst, ot, tt):
                pj = psum.tile([128, 512], F32, tag="mm", bufs=2)
                nc.tensor.matmul(
                    pj,
                    lhsT=wps[p][:, ot * 128:(ot + 1) * 128],
                    rhs=outTn[:, p, tt * 512:(tt + 1) * 512],
                    start=True,
                    stop=True,
                )
                o_sb = evac.tile([128, 512], F32, tag="osb")
                nc.vector.tensor_copy(o_sb, pj)
                nc.sync.dma_start(
                    out=dst[ot * 128:(ot + 1) * 128, tt * 512:(tt + 1) * 512],
                    in_=o_sb,
                )

            qk1 = [(w_s, dst, nt) for w_s, dst in ((wqs, qT), (wks, kT))
                   for nt in range(IT)]
            proj_q = []  # (p, ot, tt) tiles whose outTn slice is complete
            for p in range(PAIRS):
                for it in range(IT):
                    pvA = psum.tile([128, 512], F32, tag="pvA", bufs=1)
                    pvB = psum.tile([128, 512], F32, tag="pvB", bufs=1)
                    sc_next = emit_scores(p, it, 0)
                    for jt in range(JT):
                        sc = sc_next
                        if jt + 1 < JT:
                            sc_next = emit_scores(p, it, jt + 1)
                        if p == 0 and it == 0:
                            emit_v_chunk(jt)  # ready before this jt's PV
                        elif p == 0 and it in (1, 2) and jt % 4 == 0:
                            w_s, dst, nt = qk1.pop(0)  # qk pair 1, 8 tiles
                            emit_qk_tile(1, w_s, dst, nt)
                        elif jt % 2 == 1 and proj_q:
                            pp, ot, tt = proj_q.pop(0)  # stream proj tiles
                            emit_proj_tile(pp, pT0 if pp == 0 else pT1, ot, tt)
                        e = exps.tile([128, 1024], F32R, tag="e")
                        nc.scalar.activation(e, sc, AF.Exp, scale=SCALE)
                        st, sp = (jt == 0), (jt == JT - 1)
                        for h, pvx in ((0, pvA), (1, pvB)):
                            off = (2 * p + h) * 65
                            nc.tensor.matmul(
                                pvx[0:65, :],
                                lhsT=v_s[:, jt, off:off + 65],
                                rhs=e[:, h * 512:(h + 1) * 512],
                                start=st,
                                stop=sp,
                            )
                    # normalize: outTn[:, p, i-tile] = pv * (1/den) bcast over rows
                    with nc.allow_low_precision(reason="f32r softmax denom"):
                        nc.vector.reciprocal(recip_pad[0:1, :], pvA[64:65, :])
                        nc.vector.reciprocal(recip_pad[64:65, :], pvB[64:65, :])
                    bc = psum.tile([128, 1024], F32, tag="sc", bufs=2)
                    nc.tensor.matmul(
                        bc[:, 0:512], lhsT=selAB, rhs=recip_pad,
                        start=True, stop=True,
                    )
                    bc_sb = evac.tile([128, 512], F32, tag="bc")
                    nc.vector.tensor_copy(bc_sb, bc[:, 0:512])
                    nc.vector.tensor_mul(
                        outTn[0:64, p, it * 512:(it + 1) * 512],
                        pvA[0:64, :], bc_sb[0:64, :],
                    )
                    nc.vector.tensor_mul(
                        outTn[64:128, p, it * 512:(it + 1) * 512],
                        pvB[0:64, :], bc_sb[64:128, :],
                    )
                    proj_q.extend((p, ot, it) for ot in range(D // 128))

            # ---- P3: drain remaining proj tiles ----
            for pp, ot, tt in proj_q:
                emit_proj_tile(pp, pT0 if pp == 0 else pT1, ot, tt)
    return nc


_NC = None


def _get_nc():
    global _NC
    if _NC is None:
        _NC = build_bass()
        _NC.finalize()
    return _NC


_ONES4 = np.ones((128, 4), np.float32)
_SELAB = np.zeros((128, 128), np.float32)
_SELAB[0, 0:64] = 1.0
_SELAB[64, 64:128] = 1.0
_ZEROS = np.zeros((128, 512), np.float32)


def make_in_maps(x, w_qkv, w_proj):
    x = np.ascontiguousarray(np.asarray(x, np.float32))
    w_qkv = np.ascontiguousarray(np.asarray(w_qkv, np.float32))
    w_proj = np.ascontiguousarray(np.asarray(w_proj, np.float32))
    xTs = [np.ascontiguousarray(x[b].T) for b in range(B)]
    in_maps = []
    for c in range(N_CORES):
        b, g = divmod(c, TP)
        h0 = g * HDIM
        in_maps.append({
            "xT": xTs[b],
            "wq": np.ascontiguousarray(w_qkv[:, h0:h0 + HDIM]),
            "wk": np.ascontiguousarray(w_qkv[:, D + h0:D + h0 + HDIM]),
            "wv": np.ascontiguousarray(w_qkv[:, 2 * D + h0:2 * D + h0 + HDIM]),
            "wp": np.ascontiguousarray(w_proj[h0:h0 + HDIM, :]),
            "ones4_c": _ONES4,
            "selAB_c": _SELAB,
            "zeros_c": _ZEROS,
        })
    return in_maps


def combine_outputs(x, results):
    x = np.asarray(x, np.float32)
    out = np.empty((B, N, D), np.float32)
    for b in range(B):
        acc = x[b].astype(np.float64)
        for g in range(TP):
            acc += results[b * TP + g]["pT0"].T
            acc += results[b * TP + g]["pT1"].T
        out[b] = acc.astype(np.float32)
    return out


def kernel(x, w_qkv, w_proj):
    nc = _get_nc()
    in_maps = make_in_maps(x, w_qkv, w_proj)
    res = run_bass_kernel_spmd(nc, in_maps, list(range(N_CORES))).results
    return combine_outputs(x, res)



# revision 11
# speedup vs baseline: 1.1657x; 1.1657x over previous
"""Multi-head self-attention (B=2, N=2048, D=1024, H=16) on 8 Trainium2 cores.

Sharding: core c -> batch b = c // 4, head group g = c % 4 (heads 4g..4g+3,
organized as 2 pairs of 2 heads).  Optimized for the Tile cost model:

  * ACT exp stream is the roofline (~133us): 128 exps of [128,1024] paced by a
    double-buffered scores PSUM rotation, emitted at top priority so the
    Activation engine never starves.
  * PV runs d-major (out[i,d]) in bf16: the stationary operand is the exp'd
    scores chunk (full 128-wide), the moving operand is v plus a ones column
    (65 wide) whose output column accumulates the softmax denominator.
    Normalize is a per-partition reciprocal + tensor_scalar_mul; XBAR
    DMA-transpose then builds outT with no PE transpose / PSUM / DVE evac.
    This halves the PE cost of PV vs the transposed-M=65 formulation.
  * All matmul operands are bf16 (inputs cast on host: DMA bytes halved);
    accumulation stays fp32 in PSUM.  proj partials stream out as bf16 via
    gpsimd casting DMA.  Host adds residual and sums 4 partials per batch.
"""

import numpy as np
import ml_dtypes

import concourse.bass as bass
import concourse.bacc as bacc
import concourse.mybir as mybir
import concourse.tile as tile
from concourse.bass_utils import run_bass_kernel_spmd

B = 2
N = 2048
D = 1024
NH = 16
DH = 64
N_CORES = 8
TP = 4                 # head-parallel ways per batch
HPC = NH // TP         # 4 heads per core
PAIRS = 2
HDIM = HPC * DH        # 256 head dims per core
SCALE = 1.0 / 8.0      # 1/sqrt(DH)

IT = N // 512          # 4 i-tiles
JT = N // 128          # 16 j-chunks
KC = D // 128          # 8 feature chunks

F32 = mybir.dt.float32
BF16 = mybir.dt.bfloat16
AF = mybir.ActivationFunctionType


def build_bass():
    nc = bacc.Bacc("TRN2", target_bir_lowering=False, debug=False)
    xq_d = nc.declare_dram_parameter("xq", [128, KC * N], BF16, isOutput=False)
    wq_d = nc.declare_dram_parameter("wq", [128, KC * HDIM], BF16, isOutput=False)
    wk_d = nc.declare_dram_parameter("wk", [128, KC * HDIM], BF16, isOutput=False)
    wv_d = nc.declare_dram_parameter("wv", [128, KC * HDIM], BF16, isOutput=False)
    wp_d = nc.declare_dram_parameter("wp", [128, 2 * D], BF16, isOutput=False)
    o_d = nc.declare_dram_parameter("o", [N, D], BF16, isOutput=True)

    with tile.TileContext(nc) as tc:
        with (
            tc.tile_pool(name="big", bufs=1) as big,
            tc.tile_pool(name="ep", bufs=10) as ep,
            tc.tile_pool(name="sb", bufs=2) as sb,
            tc.tile_pool(name="psum", bufs=1, space="PSUM") as psum,
        ):
            # ---- warm the exp table off the critical path
            wt = big.tile([1, 2], BF16, tag="warm")
            nc.vector.memset(wt[0:1, 0:1], 0.5)
            wo = big.tile([1, 2], BF16, tag="warmo")
            nc.scalar.activation(wo[0:1, 0:1], wt[0:1, 0:1], AF.Exp)

            # ---- persistent tiles
            xq = big.tile([128, KC * N], BF16, tag="xq")
            wq = big.tile([128, KC * HDIM], BF16, tag="wq")
            wk = big.tile([128, KC * HDIM], BF16, tag="wk")
            wv = big.tile([128, KC * HDIM], BF16, tag="wv")
            wp2 = big.tile([128, 2 * D], BF16, tag="wp")
            qT = [big.tile([128, N], BF16, tag=f"qT{p}", name=f"qT{p}") for p in range(PAIRS)]
            kT = [big.tile([128, N], BF16, tag=f"kT{p}", name=f"kT{p}") for p in range(PAIRS)]
            v_s = big.tile([128, JT * HPC * 65], BF16, tag="v")
            outT2 = big.tile([128, 2 * N], BF16, tag="outT")

            xqr = xq.rearrange("p (c n) -> p c n", c=KC)
            wqr = wq.rearrange("p (q c m) -> p q c m", q=2, c=KC)
            wkr = wk.rearrange("p (q c m) -> p q c m", q=2, c=KC)
            wvr = wv.rearrange("p (c m) -> p c m", c=KC)
            wpr = wp2.rearrange("p (i n) -> p i n", i=2)
            v_sr = v_s.rearrange("p (t h c) -> p t h c", t=JT, h=HPC)
            outTr = outT2.rearrange("p (i n) -> p i n", i=2)

            # ---- input DMAs (sync queue; global DMA serializes in issue order)
            nc.sync.dma_start(out=wk[:, 0:KC * 128], in_=wk_d[:, 0:KC * 128])
            nc.sync.dma_start(out=wq[:, 0:KC * 128], in_=wq_d[:, 0:KC * 128])
            for q in range(4):
                for c in range(KC):
                    base = c * N + q * 512
                    nc.sync.dma_start(
                        out=xq[:, base:base + 512], in_=xq_d[:, base:base + 512]
                    )
                if q == 0:
                    nc.sync.dma_start(out=wv, in_=wv_d[:, :])
                    nc.sync.dma_start(out=wk[:, KC * 128:], in_=wk_d[:, KC * 128:])
                    nc.sync.dma_start(out=wq[:, KC * 128:], in_=wq_d[:, KC * 128:])
            nc.sync.dma_start(out=wp2, in_=wp_d[:, :])
            # denominator ones column (65th col of each v block)
            nc.vector.memset(v_sr[:, :, :, 64:65], 1.0)

            # ---- qkv emitters
            def emit_qk(p, which, it):
                w, dst = (wqr, qT[p]) if which == "q" else (wkr, kT[p])
                ps = psum.tile([128, 512], F32, tag="mm", bufs=2)
                for c in range(KC):
                    nc.tensor.matmul(
                        ps,
                        lhsT=w[:, p, c, :],
                        rhs=xqr[:, c, it * 512:(it + 1) * 512],
                        start=(c == 0),
                        stop=(c == KC - 1),
                    )
                nc.vector.tensor_copy(dst[:, it * 512:(it + 1) * 512], ps)

            def emit_v(t):
                ps = psum.tile([128, 512], F32, tag="mm", bufs=2)
                for c in range(KC):
                    nc.tensor.matmul(
                        ps[:, 0:HDIM],
                        lhsT=xqr[:, c, t * 128:(t + 1) * 128],
                        rhs=wvr[:, c, :],
                        start=(c == 0),
                        stop=(c == KC - 1),
                    )
                nc.vector.tensor_copy(
                    v_sr[:, t, :, 0:64],
                    ps[:, 0:HDIM].rearrange("p (h d) -> p h d", d=64),
                )

            emit_qk(0, "k", 0)
            emit_qk(0, "q", 0)

            deferred = []
            deferred += [("qk", 0, "k", 1), ("qk", 0, "k", 2), ("qk", 0, "k", 3)]
            deferred += [("qk", 0, "q", 1), ("qk", 0, "q", 2), ("qk", 0, "q", 3)]
            deferred += [("qk", 1, "k", 0), ("qk", 1, "k", 1), ("qk", 1, "q", 0)]
            deferred += [("qk", 1, "k", 2), ("qk", 1, "q", 1), ("qk", 1, "k", 3)]
            deferred += [("qk", 1, "q", 2), ("qk", 1, "q", 3)]
            deferred.reverse()  # pop() from the end

            b3 = [600_000]

            def emit_b3(fn, *args):
                save = tc.cur_priority
                tc.cur_priority = b3[0]
                fn(*args)
                b3[0] = tc.cur_priority
                tc.cur_priority = save

            def pop_deferred():
                if not deferred:
                    return
                _, p, which, it = deferred.pop()
                emit_b3(emit_qk, p, which, it)

            # ---- the exp-paced stream
            for p in range(PAIRS):
                for it in range(IT):
                    accs = None
                    for jt in range(JT):
                        s = (p * IT + it) * JT + jt
                        tc.cur_priority = 100_000 + s * 20
                        # scores^T [j, i] for both heads of the pair (K=64)
                        sct = psum.tile([128, 1024], F32, tag="sc", bufs=2)
                        for h in range(2):
                            nc.tensor.matmul(
                                sct[:, h * 512:(h + 1) * 512],
                                lhsT=kT[p][h * 64:(h + 1) * 64, jt * 128:(jt + 1) * 128],
                                rhs=qT[p][h * 64:(h + 1) * 64, it * 512:(it + 1) * 512],
                                start=True,
                                stop=True,
                            )
                        e1 = ep.tile([128, 1024], BF16, tag="e")
                        nc.scalar.activation(e1, sct, AF.Exp, scale=SCALE)

                        tc.cur_priority = 300_000 + s * 20
                        if p == 0 and it == 0:
                            emit_v(jt)  # must precede the PV that reads it
                        if jt == 0:
                            accs = (
                                psum.tile([128, 260], F32, tag="acc", bufs=2, name="accA"),
                                psum.tile([128, 260], F32, tag="acc", bufs=2, name="accB"),
                            )
                        # PV d-major: stationary e chunk (128 i's), moving v|1
                        for h in range(2):
                            for b in range(4):
                                col = ((b % 2) * 2 + h) * 65
                                nc.tensor.matmul(
                                    accs[b // 2][:, col:col + 65],
                                    lhsT=e1[:, h * 512 + b * 128: h * 512 + (b + 1) * 128],
                                    rhs=v_sr[:, jt, 2 * p + h, :],
                                    start=(jt == 0),
                                    stop=(jt == JT - 1),
                                )
                        pop_deferred()

                    # ---- normalize (per-partition recip x 8 groups) ----
                    tc.cur_priority = 300_000 + ((p * IT + it) * JT + JT) * 20 - 10
                    rc = sb.tile([128, 8], F32, tag="rc")
                    for half in range(2):
                        nc.vector.reciprocal(
                            rc[:, half * 4:(half + 1) * 4],
                            accs[half].rearrange("p (g c) -> p g c", c=65)[:, :, 64],
                        )
                    outn = sb.tile([128, 512], BF16, tag="outn")
                    for half in range(2):
                        for g in range(4):
                            b = half * 2 + g // 2
                            h = g % 2
                            nc.vector.tensor_scalar_mul(
                                out=outn[:, b * 128 + h * 64: b * 128 + (h + 1) * 64],
                                in0=accs[half][:, g * 65: g * 65 + 64],
                                scalar1=rc[:, half * 4 + g: half * 4 + g + 1],
                            )
                    # XBAR transpose [i,(h d)] -> [(h d), i] straight into outT
                    for b in range(4):
                        nc.sync.dma_start(
                            out=outTr[:, p, it * 512 + b * 128: it * 512 + (b + 1) * 128],
                            in_=outn[:, b * 128:(b + 1) * 128],
                            transpose=True,
                        )

                    # ---- proj for this i-tile once both pairs are in outT
                    if p == 1:
                        tc.cur_priority = b3[0]
                        for bloc in range(4):
                            ic = it * 4 + bloc
                            for ds in range(2):
                                # on the last i-tile alternate psum tags (the
                                # sc rotation is free once exps end) so the
                                # mm+DMA chains of the tail overlap.
                                if it == IT - 1 and (bloc * 2 + ds) % 2 == 1:
                                    pj = psum.tile([128, 1024], F32, tag="sc", bufs=2, name="pjsc")[:, 0:512]
                                else:
                                    pj = psum.tile([128, 512], F32, tag="mm", bufs=2)
                                for pp in range(2):
                                    nc.tensor.matmul(
                                        pj,
                                        lhsT=outTr[:, pp, ic * 128:(ic + 1) * 128],
                                        rhs=wpr[:, pp, ds * 512:(ds + 1) * 512],
                                        start=(pp == 0),
                                        stop=(pp == 1),
                                    )
                                osb = sb.tile([128, 512], BF16, tag="osb")
                                nc.vector.tensor_copy(osb, pj)
                                nc.sync.dma_start(
                                    out=o_d[ic * 128:(ic + 1) * 128, ds * 512:(ds + 1) * 512],
                                    in_=osb,
                                )
                        b3[0] = tc.cur_priority
    return nc


_NC = None


def _get_nc():
    global _NC
    if _NC is None:
        _NC = build_bass()
        _NC.finalize()
    return _NC


def _chunk_pack(mat):
    """[1024, M] -> [128, 8*M]: feature chunk c at cols c*M..(c+1)*M."""
    M = mat.shape[1]
    return np.ascontiguousarray(
        mat.reshape(KC, 128, M).transpose(1, 0, 2).reshape(128, KC * M)
    )


def _pair_pack(mat):
    """[1024, 256] -> [128, 2*8*128]: pair-major (pair, chunk, 128 cols)."""
    return np.ascontiguousarray(
        mat.reshape(KC, 128, 2, 128).transpose(1, 2, 0, 3).reshape(128, 2 * KC * 128)
    )


def make_in_maps(x, w_qkv, w_proj):
    bf = ml_dtypes.bfloat16
    x = np.asarray(x, np.float32)
    w_qkv = np.asarray(w_qkv, np.float32)
    w_proj = np.asarray(w_proj, np.float32)
    in_maps = []
    for c in range(N_CORES):
        b, g = divmod(c, TP)
        h0 = g * HDIM
        xT = np.ascontiguousarray(x[b].T)
        in_maps.append({
            "xq": _chunk_pack(xT).astype(bf),
            "wq": _pair_pack(w_qkv[:, h0:h0 + HDIM]).astype(bf),
            "wk": _pair_pack(w_qkv[:, D + h0:D + h0 + HDIM]).astype(bf),
            "wv": _chunk_pack(w_qkv[:, 2 * D + h0:2 * D + h0 + HDIM]).astype(bf),
            "wp": np.ascontiguousarray(
                w_proj[h0:h0 + HDIM, :].reshape(2, 128, D).transpose(1, 0, 2).reshape(128, 2 * D)
            ).astype(bf),
        })
    return in_maps


def combine_outputs(x, results):
    x = np.asarray(x, np.float32)
    out = np.empty((B, N, D), np.float32)
    for b in range(B):
        acc = x[b].astype(np.float64)
        for g in range(TP):
            acc += results[b * TP + g]["o"].astype(np.float32)
        out[b] = acc.astype(np.float32)
    return out


def kernel(x, w_qkv, w_proj):
    nc = _get_nc()
    in_maps = make_in_maps(x, w_qkv, w_proj)
    res = run_bass_kernel_spmd(nc, in_maps, list(range(N_CORES))).results
    return combine_outputs(x, res)


# revision 12
# speedup vs baseline: 1.3260x; 1.1375x over previous
"""Multi-head self-attention (B=2, N=2048, D=1024, H=16) on 8 Trainium2 cores.

Sharding: core c -> batch b = c // 4, head group g = c % 4 (heads 4g..4g+3,
organized as 2 pairs of 2 heads).  Optimized for the Tile cost model:

  * ACT exp stream is the roofline (~133us): 128 exps of [128,1024] paced by a
    double-buffered scores PSUM rotation, emitted at top priority so the
    Activation engine never starves.
  * PV runs d-major (out[i,d]) in bf16: the stationary operand is the exp'd
    scores chunk (full 128-wide), the moving operand is v plus a ones column
    (65 wide) whose output column accumulates the softmax denominator.
    Normalize is a per-partition reciprocal + tensor_scalar_mul; XBAR
    DMA-transpose then builds outT with no PE transpose / PSUM / DVE evac.
    This halves the PE cost of PV vs the transposed-M=65 formulation.
  * All matmul operands are bf16 (inputs cast on host: DMA bytes halved);
    accumulation stays fp32 in PSUM.  proj partials stream out as bf16 via
    gpsimd casting DMA.  Host adds residual and sums 4 partials per batch.
"""

import numpy as np
import ml_dtypes

import concourse.bass as bass
import concourse.bacc as bacc
import concourse.mybir as mybir
import concourse.tile as tile
from concourse.bass_utils import run_bass_kernel_spmd

B = 2
N = 2048
D = 1024
NH = 16
DH = 64
N_CORES = 8
TP = 4                 # head-parallel ways per batch
HPC = NH // TP         # 4 heads per core
PAIRS = 2
HDIM = HPC * DH        # 256 head dims per core
SCALE = 1.0 / 8.0      # 1/sqrt(DH)

IT = N // 512          # 4 i-tiles
JT = N // 128          # 16 j-chunks
KC = D // 128          # 8 feature chunks

F32 = mybir.dt.float32
BF16 = mybir.dt.bfloat16
AF = mybir.ActivationFunctionType


def build_bass():
    nc = bacc.Bacc("TRN2", target_bir_lowering=False, debug=False)
    xq_d = nc.declare_dram_parameter("xq", [128, KC * N], BF16, isOutput=False)
    wq_d = nc.declare_dram_parameter("wq", [128, KC * HDIM], BF16, isOutput=False)
    wk_d = nc.declare_dram_parameter("wk", [128, KC * HDIM], BF16, isOutput=False)
    wv_d = nc.declare_dram_parameter("wv", [128, KC * HDIM], BF16, isOutput=False)
    wp_d = nc.declare_dram_parameter("wp", [128, 2 * D], BF16, isOutput=False)
    o_d = nc.declare_dram_parameter("o", [N, D], BF16, isOutput=True)

    with tile.TileContext(nc) as tc:
        with (
            tc.tile_pool(name="big", bufs=1) as big,
            tc.tile_pool(name="ep", bufs=28) as ep,
            tc.tile_pool(name="sb", bufs=2) as sb,
            tc.tile_pool(name="psum", bufs=1, space="PSUM") as psum,
        ):
            # ---- warm the exp table off the critical path
            wt = big.tile([1, 2], BF16, tag="warm")
            nc.vector.memset(wt[0:1, 0:1], 0.5)
            wo = big.tile([1, 2], BF16, tag="warmo")
            nc.scalar.activation(wo[0:1, 0:1], wt[0:1, 0:1], AF.Exp)

            # ---- persistent tiles
            xq = big.tile([128, KC * N], BF16, tag="xq")
            wq = big.tile([128, KC * HDIM], BF16, tag="wq")
            wk = big.tile([128, KC * HDIM], BF16, tag="wk")
            wv = big.tile([128, KC * HDIM], BF16, tag="wv")
            wp2 = big.tile([128, 2 * D], BF16, tag="wp")
            qT = [big.tile([128, N], BF16, tag=f"qT{p}", name=f"qT{p}") for p in range(PAIRS)]
            kT = [big.tile([128, N], BF16, tag=f"kT{p}", name=f"kT{p}") for p in range(PAIRS)]
            v_s = big.tile([128, JT * HPC * 65], BF16, tag="v")
            outT2 = big.tile([128, 2 * N], BF16, tag="outT")

            xqr = xq.rearrange("p (c n) -> p c n", c=KC)
            wqr = wq.rearrange("p (q c m) -> p q c m", q=2, c=KC)
            wkr = wk.rearrange("p (q c m) -> p q c m", q=2, c=KC)
            wvr = wv.rearrange("p (c m) -> p c m", c=KC)
            wpr = wp2.rearrange("p (i n) -> p i n", i=2)
            v_sr = v_s.rearrange("p (t h c) -> p t h c", t=JT, h=HPC)
            outTr = outT2.rearrange("p (i n) -> p i n", i=2)

            # ---- input DMAs (sync queue; global DMA serializes in issue order)
            nc.sync.dma_start(out=wk[:, 0:KC * 128], in_=wk_d[:, 0:KC * 128])
            nc.sync.dma_start(out=wq[:, 0:KC * 128], in_=wq_d[:, 0:KC * 128])
            for q in range(4):
                for c in range(KC):
                    base = c * N + q * 512
                    nc.sync.dma_start(
                        out=xq[:, base:base + 512], in_=xq_d[:, base:base + 512]
                    )
                if q == 0:
                    nc.sync.dma_start(out=wv, in_=wv_d[:, :])
                    nc.sync.dma_start(out=wk[:, KC * 128:], in_=wk_d[:, KC * 128:])
                    nc.sync.dma_start(out=wq[:, KC * 128:], in_=wq_d[:, KC * 128:])
            nc.sync.dma_start(out=wp2, in_=wp_d[:, :])
            # denominator ones column (65th col of each v block)
            nc.vector.memset(v_sr[:, :, :, 64:65], 1.0)

            # ---- qkv emitters
            def emit_qk(p, which, it):
                w, dst = (wqr, qT[p]) if which == "q" else (wkr, kT[p])
                ps = psum.tile([128, 512], F32, tag="mm", bufs=2)
                for c in range(KC):
                    nc.tensor.matmul(
                        ps,
                        lhsT=w[:, p, c, :],
                        rhs=xqr[:, c, it * 512:(it + 1) * 512],
                        start=(c == 0),
                        stop=(c == KC - 1),
                    )
                nc.vector.tensor_copy(dst[:, it * 512:(it + 1) * 512], ps)

            def emit_v(t):
                ps = psum.tile([128, 512], F32, tag="mm", bufs=2)
                for c in range(KC):
                    nc.tensor.matmul(
                        ps[:, 0:HDIM],
                        lhsT=xqr[:, c, t * 128:(t + 1) * 128],
                        rhs=wvr[:, c, :],
                        start=(c == 0),
                        stop=(c == KC - 1),
                    )
                nc.vector.tensor_copy(
                    v_sr[:, t, :, 0:64],
                    ps[:, 0:HDIM].rearrange("p (h d) -> p h d", d=64),
                )

            # interleaved kT/qT it0 chains: both ready ~1 chain earlier
            ps_k0 = psum.tile([128, 512], F32, tag="mm", bufs=2)
            ps_q0 = psum.tile([128, 512], F32, tag="mm", bufs=2)
            for c in range(KC):
                for w, ps in ((wkr, ps_k0), (wqr, ps_q0)):
                    nc.tensor.matmul(
                        ps,
                        lhsT=w[:, 0, c, :],
                        rhs=xqr[:, c, 0:512],
                        start=(c == 0),
                        stop=(c == KC - 1),
                    )
            nc.vector.tensor_copy(kT[0][:, 0:512], ps_k0)
            nc.vector.tensor_copy(qT[0][:, 0:512], ps_q0)

            deferred = []
            deferred += [(0, "qk", 0, "k", 1), (1, "qk", 0, "k", 2), (2, "qk", 0, "k", 3)]
            deferred += [(3, "qk", 0, "q", 1), (4, "qk", 0, "q", 2), (5, "qk", 0, "q", 3)]
            deferred += [(16, "qk", 1, "k", 0), (18, "qk", 1, "k", 1), (20, "qk", 1, "q", 0)]
            deferred += [(22, "qk", 1, "k", 2), (24, "qk", 1, "q", 1), (26, "qk", 1, "k", 3)]
            deferred += [(28, "qk", 1, "q", 2), (30, "qk", 1, "q", 3)]
            deferred.reverse()  # pop() from the end

            b3 = [600_000]

            def emit_b3(fn, *args):
                save = tc.cur_priority
                tc.cur_priority = b3[0]
                fn(*args)
                b3[0] = tc.cur_priority
                tc.cur_priority = save

            def pop_deferred(s):
                while deferred and deferred[-1][0] <= s:
                    _, _, p, which, it = deferred.pop()
                    emit_b3(emit_qk, p, which, it)

            # ---- the exp-paced stream
            for p in range(PAIRS):
                for it in range(IT):
                    accs = None
                    for jt in range(JT):
                        s = (p * IT + it) * JT + jt
                        tc.cur_priority = 100_000 + s * 20
                        # scores^T [j, i] for both heads of the pair (K=64)
                        sct = psum.tile([128, 1024], F32, tag="sc", bufs=2)
                        for h in range(2):
                            nc.tensor.matmul(
                                sct[:, h * 512:(h + 1) * 512],
                                lhsT=kT[p][h * 64:(h + 1) * 64, jt * 128:(jt + 1) * 128],
                                rhs=qT[p][h * 64:(h + 1) * 64, it * 512:(it + 1) * 512],
                                start=True,
                                stop=True,
                            )
                        e1 = ep.tile([128, 1024], BF16, tag="e")
                        nc.scalar.activation(e1, sct, AF.Exp, scale=SCALE)

                        tc.cur_priority = 300_000 + s * 20
                        if p == 0 and it == 0:
                            emit_v(jt)  # must precede the PV that reads it
                        if jt == 0:
                            accs = (
                                psum.tile([128, 260], F32, tag="acc", bufs=2, name="accA"),
                                psum.tile([128, 260], F32, tag="acc", bufs=2, name="accB"),
                            )
                        # PV d-major: stationary e chunk (128 i's), moving v|1
                        for h in range(2):
                            for b in range(4):
                                col = ((b % 2) * 2 + h) * 65
                                nc.tensor.matmul(
                                    accs[b // 2][:, col:col + 65],
                                    lhsT=e1[:, h * 512 + b * 128: h * 512 + (b + 1) * 128],
                                    rhs=v_sr[:, jt, 2 * p + h, :],
                                    start=(jt == 0),
                                    stop=(jt == JT - 1),
                                )
                        pop_deferred(s)

                    # ---- normalize (per-partition recip x 8 groups) ----
                    tc.cur_priority = 300_000 + ((p * IT + it) * JT + JT) * 20 - 10
                    rc = sb.tile([128, 8], F32, tag="rc")
                    for half in range(2):
                        nc.vector.reciprocal(
                            rc[:, half * 4:(half + 1) * 4],
                            accs[half].rearrange("p (g c) -> p g c", c=65)[:, :, 64],
                        )
                    outn = sb.tile([128, 512], BF16, tag="outn")
                    for half in range(2):
                        for g in range(4):
                            b = half * 2 + g // 2
                            h = g % 2
                            nc.vector.tensor_scalar_mul(
                                out=outn[:, b * 128 + h * 64: b * 128 + (h + 1) * 64],
                                in0=accs[half][:, g * 65: g * 65 + 64],
                                scalar1=rc[:, half * 4 + g: half * 4 + g + 1],
                            )
                    # XBAR transpose [i,(h d)] -> [(h d), i] straight into outT
                    for b in range(4):
                        nc.sync.dma_start(
                            out=outTr[:, p, it * 512 + b * 128: it * 512 + (b + 1) * 128],
                            in_=outn[:, b * 128:(b + 1) * 128],
                            transpose=True,
                        )

                    # ---- proj for this i-tile once both pairs are in outT
                    if p == 1:
                        tc.cur_priority = b3[0]
                        for bloc in range(4):
                            ic = it * 4 + bloc
                            for ds in range(2):
                                # on the last i-tile alternate psum tags (the
                                # sc rotation is free once exps end) so the
                                # mm+DMA chains of the tail overlap.
                                if it == IT - 1 and (bloc * 2 + ds) % 2 == 1:
                                    pj = psum.tile([128, 1024], F32, tag="sc", bufs=2, name="pjsc")[:, 0:512]
                                else:
                                    pj = psum.tile([128, 512], F32, tag="mm", bufs=2)
                                for pp in range(2):
                                    nc.tensor.matmul(
                                        pj,
                                        lhsT=outTr[:, pp, ic * 128:(ic + 1) * 128],
                                        rhs=wpr[:, pp, ds * 512:(ds + 1) * 512],
                                        start=(pp == 0),
                                        stop=(pp == 1),
                                    )
                                osb = sb.tile([128, 512], BF16, tag="osb")
                                nc.vector.tensor_copy(osb, pj)
                                nc.sync.dma_start(
                                    out=o_d[ic * 128:(ic + 1) * 128, ds * 512:(ds + 1) * 512],
                                    in_=osb,
                                )
                        b3[0] = tc.cur_priority
    return nc


_NC = None


def _get_nc():
    global _NC
    if _NC is None:
        _NC = build_bass()
        _NC.finalize()
    return _NC


def _chunk_pack(mat):
    """[1024, M] -> [128, 8*M]: feature chunk c at cols c*M..(c+1)*M."""
    M = mat.shape[1]
    return np.ascontiguousarray(
        mat.reshape(KC, 128, M).transpose(1, 0, 2).reshape(128, KC * M)
    )


def _pair_pack(mat):
    """[1024, 256] -> [128, 2*8*128]: pair-major (pair, chunk, 128 cols)."""
    return np.ascontiguousarray(
        mat.reshape(KC, 128, 2, 128).transpose(1, 2, 0, 3).reshape(128, 2 * KC * 128)
    )


def make_in_maps(x, w_qkv, w_proj):
    bf = ml_dtypes.bfloat16
    x = np.asarray(x, np.float32)
    w_qkv = np.asarray(w_qkv, np.float32)
    w_proj = np.asarray(w_proj, np.float32)
    in_maps = []
    for c in range(N_CORES):
        b, g = divmod(c, TP)
        h0 = g * HDIM
        xT = np.ascontiguousarray(x[b].T)
        in_maps.append({
            "xq": _chunk_pack(xT).astype(bf),
            "wq": _pair_pack(w_qkv[:, h0:h0 + HDIM]).astype(bf),
            "wk": _pair_pack(w_qkv[:, D + h0:D + h0 + HDIM]).astype(bf),
            "wv": _chunk_pack(w_qkv[:, 2 * D + h0:2 * D + h0 + HDIM]).astype(bf),
            "wp": np.ascontiguousarray(
                w_proj[h0:h0 + HDIM, :].reshape(2, 128, D).transpose(1, 0, 2).reshape(128, 2 * D)
            ).astype(bf),
        })
    return in_maps


def combine_outputs(x, results):
    x = np.asarray(x, np.float32)
    out = np.empty((B, N, D), np.float32)
    for b in range(B):
        acc = x[b].astype(np.float64)
        for g in range(TP):
            acc += results[b * TP + g]["o"].astype(np.float32)
        out[b] = acc.astype(np.float32)
    return out


def kernel(x, w_qkv, w_proj):
    nc = _get_nc()
    in_maps = make_in_maps(x, w_qkv, w_proj)
    res = run_bass_kernel_spmd(nc, in_maps, list(range(N_CORES))).results
    return combine_outputs(x, res)


# revision 13
# speedup vs baseline: 1.3798x; 1.0405x over previous
"""Multi-head self-attention (B=2, N=2048, D=1024, H=16) on 8 Trainium2 cores.

Sharding: core c -> batch b = c // 4, head group g = c % 4 (heads 4g..4g+3,
organized as 2 pairs of 2 heads).  Optimized for the Tile cost model:

  * ACT exp stream is the roofline (~133us): 128 exps of [128,1024] paced by a
    double-buffered scores PSUM rotation, emitted at top priority so the
    Activation engine never starves.
  * PV runs d-major (out[i,d]) in bf16: the stationary operand is the exp'd
    scores chunk (full 128-wide), the moving operand is v plus a ones column
    (65 wide) whose output column accumulates the softmax denominator.
    Normalize is a per-partition reciprocal + tensor_scalar_mul; XBAR
    DMA-transpose then builds outT with no PE transpose / PSUM / DVE evac.
    This halves the PE cost of PV vs the transposed-M=65 formulation.
  * All matmul operands are bf16 (inputs cast on host: DMA bytes halved);
    accumulation stays fp32 in PSUM.  proj partials stream out as bf16 via
    gpsimd casting DMA.  Host adds residual and sums 4 partials per batch.
"""

import numpy as np
import ml_dtypes

import concourse.bass as bass
import concourse.bacc as bacc
import concourse.mybir as mybir
import concourse.tile as tile
from concourse.bass_utils import run_bass_kernel_spmd

B = 2
N = 2048
D = 1024
NH = 16
DH = 64
N_CORES = 8
TP = 4                 # head-parallel ways per batch
HPC = NH // TP         # 4 heads per core
PAIRS = 2
HDIM = HPC * DH        # 256 head dims per core
SCALE = 1.0 / 8.0      # 1/sqrt(DH)

IT = N // 512          # 4 i-tiles
JT = N // 128          # 16 j-chunks
KC = D // 128          # 8 feature chunks

F32 = mybir.dt.float32
BF16 = mybir.dt.bfloat16
AF = mybir.ActivationFunctionType


def build_bass():
    nc = bacc.Bacc("TRN2", target_bir_lowering=False, debug=False)
    xq_d = nc.declare_dram_parameter("xq", [128, KC * N], BF16, isOutput=False)
    wq_d = nc.declare_dram_parameter("wq", [128, KC * HDIM], BF16, isOutput=False)
    wk_d = nc.declare_dram_parameter("wk", [128, KC * HDIM], BF16, isOutput=False)
    wv_d = nc.declare_dram_parameter("wv", [128, KC * HDIM], BF16, isOutput=False)
    wp_d = nc.declare_dram_parameter("wp", [128, 2 * D], BF16, isOutput=False)
    o_d = nc.declare_dram_parameter("o", [N, D], BF16, isOutput=True)

    with tile.TileContext(nc) as tc:
        with (
            tc.tile_pool(name="big", bufs=1) as big,
            tc.tile_pool(name="ep", bufs=28) as ep,
            tc.tile_pool(name="sb", bufs=2) as sb,
            tc.tile_pool(name="psum", bufs=1, space="PSUM") as psum,
        ):
            # ---- warm the exp table off the critical path
            wt = big.tile([1, 2], BF16, tag="warm")
            nc.vector.memset(wt[0:1, 0:1], 0.5)
            wo = big.tile([1, 2], BF16, tag="warmo")
            nc.scalar.activation(wo[0:1, 0:1], wt[0:1, 0:1], AF.Exp)
            # ---- PE pstate warm-up: keep the tensor engine busy through the
            # ramp window so the first real matmuls run at full clock.
            dmy = big.tile([1, 256], BF16, tag="dmy")
            nc.vector.memset(dmy[0:1, :], 0.25)
            dacc = psum.tile([128, 260], F32, tag="acc", bufs=2, name="dacc")
            for _ in range(20):
                nc.tensor.matmul(
                    dacc[0:1, 0:256], lhsT=dmy[0:1, 0:1], rhs=dmy[0:1, :],
                    start=True, stop=True,
                )

            # ---- persistent tiles
            xq = big.tile([128, KC * N], BF16, tag="xq")
            wq = big.tile([128, KC * HDIM], BF16, tag="wq")
            wk = big.tile([128, KC * HDIM], BF16, tag="wk")
            wv = big.tile([128, KC * HDIM], BF16, tag="wv")
            wp2 = big.tile([128, 2 * D], BF16, tag="wp")
            qT = [big.tile([128, N], BF16, tag=f"qT{p}", name=f"qT{p}") for p in range(PAIRS)]
            kT = [big.tile([128, N], BF16, tag=f"kT{p}", name=f"kT{p}") for p in range(PAIRS)]
            v_s = big.tile([128, JT * HPC * 65], BF16, tag="v")
            outT2 = big.tile([128, 2 * N], BF16, tag="outT")

            xqr = xq.rearrange("p (c n) -> p c n", c=KC)
            wqr = wq.rearrange("p (q c m) -> p q c m", q=2, c=KC)
            wkr = wk.rearrange("p (q c m) -> p q c m", q=2, c=KC)
            wvr = wv.rearrange("p (c m) -> p c m", c=KC)
            wpr = wp2.rearrange("p (i n) -> p i n", i=2)
            v_sr = v_s.rearrange("p (t h c) -> p t h c", t=JT, h=HPC)
            outTr = outT2.rearrange("p (i n) -> p i n", i=2)

            # ---- input DMAs (sync queue; global DMA serializes in issue order)
            nc.sync.dma_start(out=wk[:, 0:KC * 128], in_=wk_d[:, 0:KC * 128])
            nc.sync.dma_start(out=wq[:, 0:KC * 128], in_=wq_d[:, 0:KC * 128])
            for q in range(4):
                for c in range(KC):
                    base = c * N + q * 512
                    nc.sync.dma_start(
                        out=xq[:, base:base + 512], in_=xq_d[:, base:base + 512]
                    )
                if q == 0:
                    nc.sync.dma_start(out=wv, in_=wv_d[:, :])
                    nc.sync.dma_start(out=wk[:, KC * 128:], in_=wk_d[:, KC * 128:])
                    nc.sync.dma_start(out=wq[:, KC * 128:], in_=wq_d[:, KC * 128:])
            nc.sync.dma_start(out=wp2, in_=wp_d[:, :])
            # denominator ones column (65th col of each v block)
            nc.vector.memset(v_sr[:, :, :, 64:65], 1.0)

            # ---- qkv emitters
            def emit_qk(p, which, it):
                w, dst = (wqr, qT[p]) if which == "q" else (wkr, kT[p])
                ps = psum.tile([128, 512], F32, tag="mm", bufs=2)
                for c in range(KC):
                    nc.tensor.matmul(
                        ps,
                        lhsT=w[:, p, c, :],
                        rhs=xqr[:, c, it * 512:(it + 1) * 512],
                        start=(c == 0),
                        stop=(c == KC - 1),
                    )
                nc.vector.tensor_copy(dst[:, it * 512:(it + 1) * 512], ps)

            def emit_v(t):
                ps = psum.tile([128, 512], F32, tag="mm", bufs=2)
                for c in range(KC):
                    nc.tensor.matmul(
                        ps[:, 0:HDIM],
                        lhsT=xqr[:, c, t * 128:(t + 1) * 128],
                        rhs=wvr[:, c, :],
                        start=(c == 0),
                        stop=(c == KC - 1),
                    )
                nc.vector.tensor_copy(
                    v_sr[:, t, :, 0:64],
                    ps[:, 0:HDIM].rearrange("p (h d) -> p h d", d=64),
                )

            # interleaved kT/qT it0 chains: both ready ~1 chain earlier
            ps_k0 = psum.tile([128, 512], F32, tag="mm", bufs=2)
            ps_q0 = psum.tile([128, 512], F32, tag="mm", bufs=2)
            for c in range(KC):
                for w, ps in ((wkr, ps_k0), (wqr, ps_q0)):
                    nc.tensor.matmul(
                        ps,
                        lhsT=w[:, 0, c, :],
                        rhs=xqr[:, c, 0:512],
                        start=(c == 0),
                        stop=(c == KC - 1),
                    )
            nc.vector.tensor_copy(kT[0][:, 0:512], ps_k0)
            nc.vector.tensor_copy(qT[0][:, 0:512], ps_q0)

            deferred = []
            deferred += [(0, "qk", 0, "k", 1), (1, "qk", 0, "k", 2), (2, "qk", 0, "k", 3)]
            deferred += [(3, "qk", 0, "q", 1), (4, "qk", 0, "q", 2), (5, "qk", 0, "q", 3)]
            deferred += [(16, "qk", 1, "k", 0), (18, "qk", 1, "k", 1), (20, "qk", 1, "q", 0)]
            deferred += [(22, "qk", 1, "k", 2), (24, "qk", 1, "q", 1), (26, "qk", 1, "k", 3)]
            deferred += [(28, "qk", 1, "q", 2), (30, "qk", 1, "q", 3)]
            deferred.reverse()  # pop() from the end

            b3 = [600_000]

            def emit_b3(fn, *args):
                save = tc.cur_priority
                tc.cur_priority = b3[0]
                fn(*args)
                b3[0] = tc.cur_priority
                tc.cur_priority = save

            def pop_deferred(s):
                while deferred and deferred[-1][0] <= s:
                    _, _, p, which, it = deferred.pop()
                    emit_b3(emit_qk, p, which, it)

            # ---- the exp-paced stream
            for p in range(PAIRS):
                for it in range(IT):
                    accs = None
                    for jt in range(JT):
                        s = (p * IT + it) * JT + jt
                        tc.cur_priority = 100_000 + s * 20
                        # scores^T [j, i] for both heads of the pair (K=64)
                        sct = psum.tile([128, 1024], F32, tag="sc", bufs=2)
                        for h in range(2):
                            nc.tensor.matmul(
                                sct[:, h * 512:(h + 1) * 512],
                                lhsT=kT[p][h * 64:(h + 1) * 64, jt * 128:(jt + 1) * 128],
                                rhs=qT[p][h * 64:(h + 1) * 64, it * 512:(it + 1) * 512],
                                start=True,
                                stop=True,
                            )
                        e1 = ep.tile([128, 1024], BF16, tag="e")
                        nc.scalar.activation(e1, sct, AF.Exp, scale=SCALE)

                        tc.cur_priority = 300_000 + s * 20
                        if p == 0 and it == 0:
                            emit_v(jt)  # must precede the PV that reads it
                        if jt == 0:
                            accs = (
                                psum.tile([128, 260], F32, tag="acc", bufs=2, name="accA"),
                                psum.tile([128, 260], F32, tag="acc", bufs=2, name="accB"),
                            )
                        # PV d-major: stationary e chunk (128 i's), moving v|1
                        for h in range(2):
                            for b in range(4):
                                col = ((b % 2) * 2 + h) * 65
                                nc.tensor.matmul(
                                    accs[b // 2][:, col:col + 65],
                                    lhsT=e1[:, h * 512 + b * 128: h * 512 + (b + 1) * 128],
                                    rhs=v_sr[:, jt, 2 * p + h, :],
                                    start=(jt == 0),
                                    stop=(jt == JT - 1),
                                )
                        pop_deferred(s)

                    # ---- normalize (per-partition recip x 8 groups) ----
                    tc.cur_priority = 300_000 + ((p * IT + it) * JT + JT) * 20 - 10
                    rc = sb.tile([128, 8], F32, tag="rc")
                    for half in range(2):
                        nc.vector.reciprocal(
                            rc[:, half * 4:(half + 1) * 4],
                            accs[half].rearrange("p (g c) -> p g c", c=65)[:, :, 64],
                        )
                    outn = sb.tile([128, 512], BF16, tag="outn")
                    tail = (p == 1 and it == IT - 1)
                    for half in range(2):
                        for g in range(4):
                            b = half * 2 + g // 2
                            h = g % 2
                            odst = outn[:, b * 128 + h * 64: b * 128 + (h + 1) * 64]
                            isrc = accs[half][:, g * 65: g * 65 + 64]
                            sc1 = rc[:, half * 4 + g: half * 4 + g + 1]
                            if tail and g % 2 == 1:
                                nc.scalar.activation(odst, isrc, AF.Copy, scale=sc1)
                            else:
                                nc.vector.tensor_scalar_mul(out=odst, in0=isrc, scalar1=sc1)
                    # XBAR transpose [i,(h d)] -> [(h d), i] straight into outT
                    for b in range(4):
                        nc.sync.dma_start(
                            out=outTr[:, p, it * 512 + b * 128: it * 512 + (b + 1) * 128],
                            in_=outn[:, b * 128:(b + 1) * 128],
                            transpose=True,
                        )

                    # ---- proj for this i-tile once both pairs are in outT
                    if p == 1:
                        tc.cur_priority = b3[0]
                        for bloc in range(4):
                            ic = it * 4 + bloc
                            for ds in range(2):
                                # on the last i-tile alternate psum tags (the
                                # sc rotation is free once exps end) so the
                                # mm+DMA chains of the tail overlap.
                                if it == IT - 1 and (bloc * 2 + ds) % 2 == 1:
                                    pj = psum.tile([128, 1024], F32, tag="sc", bufs=2, name="pjsc")[:, 0:512]
                                else:
                                    pj = psum.tile([128, 512], F32, tag="mm", bufs=2)
                                for pp in range(2):
                                    nc.tensor.matmul(
                                        pj,
                                        lhsT=outTr[:, pp, ic * 128:(ic + 1) * 128],
                                        rhs=wpr[:, pp, ds * 512:(ds + 1) * 512],
                                        start=(pp == 0),
                                        stop=(pp == 1),
                                    )
                                osb = sb.tile([128, 512], BF16, tag="osb", bufs=6)
                                if it == IT - 1 and ds == 1:
                                    nc.scalar.activation(osb, pj, AF.Copy)
                                else:
                                    nc.vector.tensor_copy(osb, pj)
                                nc.sync.dma_start(
                                    out=o_d[ic * 128:(ic + 1) * 128, ds * 512:(ds + 1) * 512],
                                    in_=osb,
                                )
                        b3[0] = tc.cur_priority
    return nc


_NC = None


def _get_nc():
    global _NC
    if _NC is None:
        _NC = build_bass()
        _NC.finalize()
    return _NC


def _chunk_pack(mat):
    """[1024, M] -> [128, 8*M]: feature chunk c at cols c*M..(c+1)*M."""
    M = mat.shape[1]
    return np.ascontiguousarray(
        mat.reshape(KC, 128, M).transpose(1, 0, 2).reshape(128, KC * M)
    )


def _pair_pack(mat):
    """[1024, 256] -> [128, 2*8*128]: pair-major (pair, chunk, 128 cols)."""
    return np.ascontiguousarray(
        mat.reshape(KC, 128, 2, 128).transpose(1, 2, 0, 3).reshape(128, 2 * KC * 128)
    )


def make_in_maps(x, w_qkv, w_proj):
    bf = ml_dtypes.bfloat16
    x = np.asarray(x, np.float32)
    w_qkv = np.asarray(w_qkv, np.float32)
    w_proj = np.asarray(w_proj, np.float32)
    in_maps = []
    for c in range(N_CORES):
        b, g = divmod(c, TP)
        h0 = g * HDIM
        xT = np.ascontiguousarray(x[b].T)
        in_maps.append({
            "xq": _chunk_pack(xT).astype(bf),
            "wq": _pair_pack(w_qkv[:, h0:h0 + HDIM]).astype(bf),
            "wk": _pair_pack(w_qkv[:, D + h0:D + h0 + HDIM]).astype(bf),
            "wv": _chunk_pack(w_qkv[:, 2 * D + h0:2 * D + h0 + HDIM]).astype(bf),
            "wp": np.ascontiguousarray(
                w_proj[h0:h0 + HDIM, :].reshape(2, 128, D).transpose(1, 0, 2).reshape(128, 2 * D)
            ).astype(bf),
        })
    return in_maps


def combine_outputs(x, results):
    x = np.asarray(x, np.float32)
    out = np.empty((B, N, D), np.float32)
    for b in range(B):
        acc = x[b].astype(np.float64)
        for g in range(TP):
            acc += results[b * TP + g]["o"].astype(np.float32)
        out[b] = acc.astype(np.float32)
    return out


def kernel(x, w_qkv, w_proj):
    nc = _get_nc()
    in_maps = make_in_maps(x, w_qkv, w_proj)
    res = run_bass_kernel_spmd(nc, in_maps, list(range(N_CORES))).results
    return combine_outputs(x, res)
